# revision 27
# baseline (speedup 1.0000x reference)
"""PixelUnshuffle(s=2) + avg-pool concat kernel for Trainium2, 8 NeuronCores.

Semantics (per image):
  out[i, j, 4c + 2b + a] = images[2i + a, 2j + b, c]   for c<3, a,b in {0,1}
  out[i, j, 12]          = mean of maps[2i:2i+2, 2j:2j+2]

Sharding: pure data-parallel over the batch dim (32 images -> 4 per core).

Per-core pipeline:
  - maps are loaded once per image (1.05 MB DMA) with a 4D access pattern
    that puts rows {2p, 2p+1, 256+2p, 256+2p+1} on partition p, so both
    height-halves are partition-aligned with the compute chunks
  - images are loaded per height-half (1.57 MB DMA), partition p holding
    input rows {2p, 2p+1} of the half
  - compute + store run at quarter granularity (height-half x width-half):
    4 strided tensor_copy ops (DVE) rearrange the image data, two adds
    (GpSimd) + a scaled copy (DVE) produce the 2x2 map mean in channel 12,
    then a 0.85 MB store DMA writes the quarter
The op is memory-bound (~30 MB/core of DMA); all engine work hides under
the DMA stream, and the fine store granularity keeps the pipeline tail
short.
"""

import numpy as np

import concourse.bacc as bacc
import concourse.mybir as mybir
from concourse.tile import TileContext
from concourse.vector_clock import ScopedClock


class SlimEntryBacc(bacc.Bacc):
    """Bacc whose construction-time preamble skips the entry all-engine
    barrier. The barrier orders the const-tile memsets and engine table
    loads against cross-engine readers; this kernel never reads the const
    tiles, and each engine's table load precedes its compute in-order, so
    the first load DMA can issue ~3 us earlier."""

    def __init__(self, *a, **kw):
        self._skip_entry_barrier = True
        super().__init__(*a, **kw)
        self._skip_entry_barrier = False

    def all_engine_barrier(self, *, sem_only: bool = False):
        if getattr(self, "_skip_entry_barrier", False):
            return
        super().all_engine_barrier(sem_only=sem_only)


class SlimExitTileContext(TileContext):
    """TileContext with a cheaper exit sequence.

    Stock Tile ends with drain + all-engine barrier + sem clears + second
    all-engine barrier (~4-6 us). The SP drain already waits on every proc's
    final tick (engines and DMA lanes), so it is sufficient to hand off
    SP -> Pool with a 2-engine barrier and let Pool clear the semaphores;
    Pool's halt orders the clears before NEFF completion, and the other
    engines simply halt after their last op.
    """

    def _drain_and_barrier(self, tick_clock, wait_clock):
        drain_inst = self.nc.sync.drain()
        wait_clock.add_sem_waits(
            drain_inst.ins, ScopedClock({None: tick_clock.global_clock})
        )
        self.nc.all_engine_barrier()
        popped = self.nc._tile_sem_poison_stack.pop()
        assert popped is self._sem_poison
        self.nc.clear_and_free_semaphores(list(self.sems.allocated().values()))
        # no trailing all-engine barrier: nothing follows this tile context,
        # Pool halts after its clears, and NEFF completion waits for all
        # engine halts - so the clears are ordered before the next execution

N_CORES = 8
B, H, W, C = 32, 512, 512, 3
S = 2
BC = B // N_CORES  # images per core
HO, WO = H // S, W // S
K = C * S * S + 1  # 13 output channels
P = 128  # SBUF partitions

_FP = mybir.dt.float32


def build_nc(bc=BC, h=H, w=W, c=C, ws=4, img_bufs=4, map_bufs=3, out_bufs=6):
    """Build the SPMD Bass program for one core handling `bc` images.

    Each image is processed as 2 height-halves; full-width chunks except the
    final half, which is split into `ws` width-chunks to shorten the
    pipeline tail (last store only waits on a quarter's compute).
    """
    k = c * S * S + 1
    assert h == 4 * P  # height-half = 2 rows per partition
    hh = h // 2  # input rows per height-half
    assert w % ws == 0 and (w // ws) % S == 0

    # Bacc (not raw Bass): its finalize() legalizes sync waits down to the
    # per-instruction caps walrus codegen enforces.
    nc = SlimEntryBacc()
    # Per-engine drains replace the skipped entry barrier's flush semantics:
    # each engine's preamble table/library load must land before its first
    # kernel op (Pool's GPSIMD ucode library in particular - without this,
    # tensor_add intermittently computes garbage). Drains run in parallel
    # (~0.5 us) with no cross-engine sync, so SP still issues the first
    # load immediately.
    for eng in (nc.sync, nc.vector, nc.scalar, nc.gpsimd, nc.tensor):
        eng.drain()
    images = nc.declare_dram_parameter("images", [bc, h, w, c], _FP, isOutput=False)
    maps = nc.declare_dram_parameter("maps", [bc, h, w, 1], _FP, isOutput=False)
    out = nc.declare_dram_parameter("out", [bc, h // S, w // S, k], _FP, isOutput=True)

    with SlimExitTileContext(nc) as tc:
        with (
            tc.tile_pool(name="pimg", bufs=img_bufs) as pimg,
            tc.tile_pool(name="pmap", bufs=map_bufs) as pmap,
            tc.tile_pool(name="pout", bufs=out_bufs) as pout,
            tc.tile_pool(name="psml", bufs=out_bufs) as psml,
        ):
            for i in range(bc):
                # whole image's maps; partition p gets rows
                # {2p, 2p+1, hh+2p, hh+2p+1} so both height-halves align
                map_tile = pmap.tile([P, 4 * w], _FP, tag="map")
                m4 = map_tile[:].rearrange("p (s rr w) -> p s rr w", s=2, rr=2)
                nc.sync.dma_start(
                    out=m4,
                    in_=maps[i].rearrange("(s p rr) w c -> p s rr (w c)", p=P, s=2),
                )

                for hp in range(2):
                    # height-half of the image; partition p <- rows {2p, 2p+1}
                    img_tile = pimg.tile([P, 2 * w * c], _FP, tag="img")
                    nc.sync.dma_start(
                        out=img_tile[:],
                        in_=images[i][hp * hh : (hp + 1) * hh].rearrange(
                            "(p f) w c -> p (f w c)", p=P
                        ),
                    )
                    v_in = img_tile[:].rearrange("p (R w c) -> p R w c", R=2, w=w)

                    cur_ws = ws if (i == bc - 1 and hp == 1) else 1
                    wch = w // cur_ws  # input cols per width-chunk
                    woch = wch // S  # output cols per width-chunk
                    for wq in range(cur_ws):
                        out_tile = pout.tile([P, woch * k], _FP, tag="out")
                        v_out = out_tile[:].rearrange("p (w k) -> p w k", w=woch)

                        # image rearrange: channel 4c+2b+a <- rows a::2,
                        # cols b::2 of this quarter
                        for a in range(S):
                            for b in range(S):
                                nc.vector.tensor_copy(
                                    out=v_out[:, :, 2 * b + a : k - 1 : 4],
                                    in_=v_in[
                                        :,
                                        a::2,
                                        wq * wch + b : (wq + 1) * wch : 2,
                                        :,
                                    ],
                                )

                        # maps 2x2 mean -> channel 12: horizontal add,
                        # vertical add (GpSimd), scaled write (DVE - keeps
                        # out_tile single-producer-engine)
                        m_sub = m4[:, hp, :, wq * wch : (wq + 1) * wch]
                        havg = psml.tile([P, 2 * woch], _FP, tag="havg")
                        h2 = havg[:].rearrange("p (rr j) -> p rr j", rr=2)
                        nc.gpsimd.tensor_add(
                            out=h2, in0=m_sub[:, :, 0::2], in1=m_sub[:, :, 1::2]
                        )
                        vavg = psml.tile([P, woch], _FP, tag="vavg")
                        nc.gpsimd.tensor_add(
                            out=vavg[:], in0=h2[:, 0, :], in1=h2[:, 1, :]
                        )
                        nc.vector.tensor_scalar_mul(v_out[:, :, k - 1], vavg[:], 0.25)

                        # store this chunk: partition p -> output row p of
                        # the half, cols [wq*woch, (wq+1)*woch). Issued on
                        # the ACT HW-DGE ring so stores (which wait on
                        # compute) never head-of-line-block the loads on
                        # the SP ring.
                        nc.scalar.dma_start(
                            out=out[i][
                                hp * P : (hp + 1) * P,
                                wq * woch : (wq + 1) * woch,
                            ].rearrange("p w k -> p (w k)"),
                            in_=out_tile[:],
                        )
    nc.finalize()
    return nc


_CACHED_NC = None


def kernel(**inputs: np.ndarray) -> np.ndarray:
    from concourse.bass_utils import run_bass_kernel_spmd

    global _CACHED_NC
    images = np.ascontiguousarray(np.asarray(inputs["images"], dtype=np.float32))
    maps = np.ascontiguousarray(np.asarray(inputs["maps"], dtype=np.float32))
    assert images.shape == (B, H, W, C) and maps.shape == (B, H, W, 1)

    if _CACHED_NC is None:
        _CACHED_NC = build_nc()
    nc = _CACHED_NC

    in_maps = [
        {"images": images[c * BC : (c + 1) * BC], "maps": maps[c * BC : (c + 1) * BC]}
        for c in range(N_CORES)
    ]
    res = run_bass_kernel_spmd(nc, in_maps, list(range(N_CORES)))
    return np.concatenate([r["out"] for r in res.results], axis=0)


# revision 29
# speedup vs baseline: 1.1648x; 1.1648x over previous
"""PixelUnshuffle(s=2) + avg-pool concat kernel for Trainium2, 8 NeuronCores.

Semantics (per image):
  out[i, j, 4c + 2b + a] = images[2i + a, 2j + b, c]   for c<3, a,b in {0,1}
  out[i, j, 12]          = mean of maps[2i:2i+2, 2j:2j+2]

Sharding: pure data-parallel over the batch dim (32 images -> 4 per core).

Per-core pipeline:
  - maps are loaded once per image (1.05 MB DMA) with a 4D access pattern
    that puts rows {2p, 2p+1, 256+2p, 256+2p+1} on partition p, so both
    height-halves are partition-aligned with the compute chunks
  - images are loaded per height-half (1.57 MB DMA), partition p holding
    input rows {2p, 2p+1} of the half
  - compute + store run at quarter granularity (height-half x width-half):
    4 strided tensor_copy ops (DVE) rearrange the image data, two adds
    (GpSimd) + a scaled copy (DVE) produce the 2x2 map mean in channel 12,
    then a 0.85 MB store DMA writes the quarter
The op is memory-bound (~30 MB/core of DMA); all engine work hides under
the DMA stream, and the fine store granularity keeps the pipeline tail
short.
"""

import numpy as np

import concourse.bacc as bacc
import concourse.mybir as mybir
from concourse.tile import TileContext
from concourse.vector_clock import ScopedClock


class SlimExitTileContext(TileContext):
    """TileContext with a cheaper exit sequence.

    Stock Tile ends with drain + all-engine barrier + sem clears + second
    all-engine barrier (~4-6 us). The SP drain already waits on every proc's
    final tick (engines and DMA lanes), so it is sufficient to hand off
    SP -> Pool with a 2-engine barrier and let Pool clear the semaphores;
    Pool's halt orders the clears before NEFF completion, and the other
    engines simply halt after their last op.
    """

    def _drain_and_barrier(self, tick_clock, wait_clock):
        drain_inst = self.nc.sync.drain()
        wait_clock.add_sem_waits(
            drain_inst.ins, ScopedClock({None: tick_clock.global_clock})
        )
        self.nc.all_engine_barrier()
        popped = self.nc._tile_sem_poison_stack.pop()
        assert popped is self._sem_poison
        self.nc.clear_and_free_semaphores(list(self.sems.allocated().values()))
        # no trailing all-engine barrier: nothing follows this tile context,
        # Pool halts after its clears, and NEFF completion waits for all
        # engine halts - so the clears are ordered before the next execution

N_CORES = 8
B, H, W, C = 32, 512, 512, 3
S = 2
BC = B // N_CORES  # images per core
HO, WO = H // S, W // S
K = C * S * S + 1  # 13 output channels
P = 128  # SBUF partitions

_FP = mybir.dt.float32


def build_nc(bc=BC, h=H, w=W, c=C, ws=4, img_bufs=4, map_bufs=3, out_bufs=6):
    """Build the SPMD Bass program for one core handling `bc` images.

    Each image is processed as 2 height-halves; full-width chunks except the
    final half, which is split into `ws` width-chunks to shorten the
    pipeline tail (last store only waits on a quarter's compute).
    """
    k = c * S * S + 1
    assert h == 4 * P  # height-half = 2 rows per partition
    hh = h // 2  # input rows per height-half
    assert w % ws == 0 and (w // ws) % S == 0

    # Bacc (not raw Bass): its finalize() legalizes sync waits down to the
    # per-instruction caps walrus codegen enforces. The construction-time
    # entry barrier is kept: skipping it races the engines' preamble
    # table/library loads (GPSIMD ucode in particular) against the first
    # kernel ops, which intermittently corrupts results on hardware.
    nc = bacc.Bacc()
    images = nc.declare_dram_parameter("images", [bc, h, w, c], _FP, isOutput=False)
    maps = nc.declare_dram_parameter("maps", [bc, h, w, 1], _FP, isOutput=False)
    out = nc.declare_dram_parameter("out", [bc, h // S, w // S, k], _FP, isOutput=True)

    with SlimExitTileContext(nc) as tc:
        with (
            tc.tile_pool(name="pimg", bufs=img_bufs) as pimg,
            tc.tile_pool(name="pmap", bufs=map_bufs) as pmap,
            tc.tile_pool(name="pout", bufs=out_bufs) as pout,
            tc.tile_pool(name="psml", bufs=out_bufs) as psml,
        ):
            for i in range(bc):
                # whole image's maps; partition p gets rows
                # {2p, 2p+1, hh+2p, hh+2p+1} so both height-halves align
                map_tile = pmap.tile([P, 4 * w], _FP, tag="map")
                m4 = map_tile[:].rearrange("p (s rr w) -> p s rr w", s=2, rr=2)
                nc.sync.dma_start(
                    out=m4,
                    in_=maps[i].rearrange("(s p rr) w c -> p s rr (w c)", p=P, s=2),
                )

                for hp in range(2):
                    # height-half of the image; partition p <- rows {2p, 2p+1}
                    img_tile = pimg.tile([P, 2 * w * c], _FP, tag="img")
                    nc.sync.dma_start(
                        out=img_tile[:],
                        in_=images[i][hp * hh : (hp + 1) * hh].rearrange(
                            "(p f) w c -> p (f w c)", p=P
                        ),
                    )
                    v_in = img_tile[:].rearrange("p (R w c) -> p R w c", R=2, w=w)

                    cur_ws = ws if (i == bc - 1 and hp == 1) else 1
                    wch = w // cur_ws  # input cols per width-chunk
                    woch = wch // S  # output cols per width-chunk
                    for wq in range(cur_ws):
                        out_tile = pout.tile([P, woch * k], _FP, tag="out")
                        v_out = out_tile[:].rearrange("p (w k) -> p w k", w=woch)

                        # image rearrange: channel 4c+2b+a <- rows a::2,
                        # cols b::2 of this quarter
                        for a in range(S):
                            for b in range(S):
                                nc.vector.tensor_copy(
                                    out=v_out[:, :, 2 * b + a : k - 1 : 4],
                                    in_=v_in[
                                        :,
                                        a::2,
                                        wq * wch + b : (wq + 1) * wch : 2,
                                        :,
                                    ],
                                )

                        # maps 2x2 mean -> channel 12: horizontal add,
                        # vertical add (GpSimd), scaled write (DVE - keeps
                        # out_tile single-producer-engine)
                        m_sub = m4[:, hp, :, wq * wch : (wq + 1) * wch]
                        havg = psml.tile([P, 2 * woch], _FP, tag="havg")
                        h2 = havg[:].rearrange("p (rr j) -> p rr j", rr=2)
                        nc.gpsimd.tensor_add(
                            out=h2, in0=m_sub[:, :, 0::2], in1=m_sub[:, :, 1::2]
                        )
                        vavg = psml.tile([P, woch], _FP, tag="vavg")
                        nc.gpsimd.tensor_add(
                            out=vavg[:], in0=h2[:, 0, :], in1=h2[:, 1, :]
                        )
                        nc.vector.tensor_scalar_mul(v_out[:, :, k - 1], vavg[:], 0.25)

                        # store this chunk: partition p -> output row p of
                        # the half, cols [wq*woch, (wq+1)*woch). Issued on
                        # the ACT HW-DGE ring so stores (which wait on
                        # compute) never head-of-line-block the loads on
                        # the SP ring.
                        nc.scalar.dma_start(
                            out=out[i][
                                hp * P : (hp + 1) * P,
                                wq * woch : (wq + 1) * woch,
                            ].rearrange("p w k -> p (w k)"),
                            in_=out_tile[:],
                        )
    nc.finalize()
    return nc


_CACHED_NC = None


def kernel(**inputs: np.ndarray) -> np.ndarray:
    from concourse.bass_utils import run_bass_kernel_spmd

    global _CACHED_NC
    images = np.ascontiguousarray(np.asarray(inputs["images"], dtype=np.float32))
    maps = np.ascontiguousarray(np.asarray(inputs["maps"], dtype=np.float32))
    assert images.shape == (B, H, W, C) and maps.shape == (B, H, W, 1)

    if _CACHED_NC is None:
        _CACHED_NC = build_nc()
    nc = _CACHED_NC

    in_maps = [
        {"images": images[c * BC : (c + 1) * BC], "maps": maps[c * BC : (c + 1) * BC]}
        for c in range(N_CORES)
    ]
    res = run_bass_kernel_spmd(nc, in_maps, list(range(N_CORES)))
    return np.concatenate([r["out"] for r in res.results], axis=0)
